# revision 27
# baseline (speedup 1.0000x reference)
"""Multi-head causal attention (B=4, C=2048, E=1024, H=16, D=64) on 8 TRN2 cores.

Sharding: batch x head-group (4 x 2). Core c handles batch c//2 and heads
(c%2)*8 .. (c%2)*8+8.  Each core computes a partial output

    Y_c = Attn(x_b; heads hg) @ W_o[hg rows]        (shape [C, E])

and the host sums the two partials per batch (row-split W_o all-reduce done
host-side since outputs are gathered anyway).

v3 structure (one software-pipelined loop):
  * all matmul operands bf16 (fp32 PSUM accumulation) - fp32r was
    power-throttled on HW; bf16 also halves LDWEIGHTS and DMA.
  * projections of q-slice j+1 and the output projection of slice j-1 are
    interleaved as PE "filler" work between attention matmul groups of
    slice j, so softmax-exp / reciprocal latency never idles the PE.
  * causal diagonal blocks restricted to the valid column range for the
    S^T matmul, the exp, and the P@V accumulation.
  * softmax denominator: the V tiles carry a ones column so the denominator
    rides in PSUM row 64 of the P@V output; per head-pair the two halves'
    rows are staged to rows {base, base+32} of a shared FR tile (one DMA
    hop for the cross-partition moves) and one DVE reciprocal covers both;
    K=1 ones-matmuls broadcast them back across partitions for the DVE
    normalize multiply.
  * output returned in bf16 (host upcasts); halves the y DMA + evict cost.
"""

import sys

if "/opt/trn_rl_repo" not in sys.path:
    sys.path.insert(0, "/opt/trn_rl_repo")

import math

import numpy as np

B, C, E, H, D = 4, 2048, 1024, 16, 64
NCORES = 8
P = 128
CS = 512  # q-slice width


def build_module(C=C, E=E, HL=H // 2, D=D, n_devices=NCORES):
    """Build the SPMD Bass module for one core's shard."""
    from contextlib import ExitStack

    import concourse.bass as bass
    import concourse.mybir as mybir
    import concourse.tile as tile

    F32 = mybir.dt.float32
    FR = mybir.dt.float32r
    BF = mybir.dt.bfloat16
    Exp = mybir.ActivationFunctionType.Exp
    MUL = mybir.AluOpType.mult
    ISGE = mybir.AluOpType.is_ge

    ET = E // P          # e-tiles
    JT = HL * D // P     # j-tiles (head pairs)
    NJ = C // CS         # q-slices
    CT = C // P          # c-tiles
    KPJ = CS // P        # kk-tiles per q-slice (4)
    scale = 1.0 / math.sqrt(D)

    nc = bass.Bass(
        "TRN2", target_bir_lowering=False, debug=False, num_devices=n_devices
    )

    xT = nc.dram_tensor("xT", [P, ET, C], BF, kind="ExternalInput").ap()
    wq_d = nc.dram_tensor("wq", [P, ET, HL * D], BF, kind="ExternalInput").ap()
    wk_d = nc.dram_tensor("wk", [P, ET, HL * D], BF, kind="ExternalInput").ap()
    wv_d = nc.dram_tensor("wv", [P, ET, HL * D], BF, kind="ExternalInput").ap()
    wo_d = nc.dram_tensor("wo", [P, JT, E], BF, kind="ExternalInput").ap()
    y_d = nc.dram_tensor("y", [CT, P, E], BF, kind="ExternalOutput").ap()

    with tile.TileContext(nc) as tc:
        with ExitStack() as ctx:
            pA = ctx.enter_context(tc.tile_pool(name="pA", bufs=1))
            psS = ctx.enter_context(tc.tile_pool(name="psS", bufs=2, space="PSUM"))
            psPV = ctx.enter_context(tc.tile_pool(name="psPV", bufs=2, space="PSUM"))
            psMM = ctx.enter_context(tc.tile_pool(name="psMM", bufs=2, space="PSUM"))
            pX = ctx.enter_context(tc.tile_pool(name="pX", bufs=2))
            pE = ctx.enter_context(tc.tile_pool(name="pE", bufs=8))
            pT = ctx.enter_context(tc.tile_pool(name="pT", bufs=2))
            pH = ctx.enter_context(tc.tile_pool(name="pH", bufs=10))

            qt = pA.tile([P, JT, C], BF, tag="qt")
            kt = pA.tile([P, JT, C], BF, tag="kt")
            v = pA.tile([P, CT, HL, D + 1], BF, tag="v")
            hdt = pA.tile([P, JT, C], BF, tag="hdt")
            ones = pA.tile([P, 64], FR, tag="ones")
            den = pA.tile([P, CS], FR, tag="den")
            den2 = pA.tile([P, CS], FR, tag="den2")
            sg = pA.tile([P, 2, CS], F32, tag="sg")
            wq = pA.tile([P, ET, HL * D], BF, tag="wq")
            wk = pA.tile([P, ET, HL * D], BF, tag="wk")
            wv = pA.tile([P, ET, HL * D], BF, tag="wv")
            wo = pA.tile([P, JT, E], BF, tag="wo")

            xts = {}

            def load_x(cs, split=False):
                xt = pX.tile([P, ET, CS], BF, tag="xt")
                csl = slice(cs * CS, (cs + 1) * CS)
                if split:
                    h = ET // 2
                    nc.sync.dma_start(xt[:, 0:h, :], xT[:, 0:h, csl])
                    nc.scalar.dma_start(xt[:, h:ET, :], xT[:, h:ET, csl])
                else:
                    nc.sync.dma_start(xt[:], xT[:, :, csl])
                xts[cs] = xt

            # the first matmul chain consumes (wq[et], x0[et]) pairs in
            # order: issue them as alternating per-et transfers on the two
            # HWDGE queues so the chain starts as soon as et=0 lands
            xt0 = pX.tile([P, ET, CS], BF, tag="xt")
            xts[0] = xt0
            for et in range(ET):
                qa, qb = (nc.sync, nc.scalar) if et % 2 == 0 else (
                    nc.scalar,
                    nc.sync,
                )
                qa.dma_start(wq[:, et, :], wq_d[:, et, :])
                qb.dma_start(xt0[:, et, :], xT[:, et, 0:CS])
            h = ET // 2
            nc.sync.dma_start(wk[:, 0:h, :], wk_d[:, 0:h, :])
            nc.scalar.dma_start(wk[:, h:ET, :], wk_d[:, h:ET, :])
            nc.sync.dma_start(wv[:, 0:h, :], wv_d[:, 0:h, :])
            nc.scalar.dma_start(wv[:, h:ET, :], wv_d[:, h:ET, :])
            nc.sync.dma_start(wo[:], wo_d)
            nc.vector.memset(ones[:].bitcast(F32), 1.0)
            nc.vector.memset(den[:].bitcast(F32), 1.0)
            nc.vector.memset(den2[:].bitcast(F32), 1.0)
            nc.vector.memset(v[:, :, :, D : D + 1], 1.0)

            def proj_units(cs):
                """Projection work for q-slice cs as a list of callables,
                each ~8 matmuls + 1 psum evict."""
                csl = slice(cs * CS, (cs + 1) * CS)

                def qk_unit(w_sb, out_t, jt):
                    def run():
                        xt = xts[cs]
                        ps = psMM.tile([P, CS], F32, tag="mm")
                        for et in range(ET):
                            nc.tensor.matmul(
                                ps[:],
                                w_sb[:, et, jt * P : (jt + 1) * P],
                                xt[:, et, :],
                                start=(et == 0),
                                stop=(et == ET - 1),
                            )
                        nc.vector.tensor_copy(out_t[:, jt, csl], ps[:])

                    return run

                def v_unit(c4):
                    def run():
                        xt = xts[cs]
                        ct = cs * KPJ + c4
                        ps = psMM.tile([P, HL, D], F32, tag="mm")
                        for et in range(ET):
                            nc.tensor.matmul(
                                ps[:],
                                xt[:, et, c4 * P : (c4 + 1) * P],
                                wv[:, et, :],
                                start=(et == 0),
                                stop=(et == ET - 1),
                            )
                        nc.vector.tensor_copy(v[:, ct, :, 0:D], ps[:])

                    return run

                units = []
                if cs == 0:
                    # slice 0 runs at t=0: Q units only need wq + x0, which
                    # stream in first; K/V weights land while they run
                    for jt in range(JT):
                        units.append(qk_unit(wq, qt, jt))
                    for jt in range(JT):
                        units.append(qk_unit(wk, kt, jt))
                else:
                    for jt in range(JT):
                        units.append(qk_unit(wq, qt, jt))
                        units.append(qk_unit(wk, kt, jt))
                for c4 in range(KPJ):
                    units.append(v_unit(c4))
                return units

            def outproj_units(jj):
                """Output projection for the c-tiles of q-slice jj."""
                FS = min(CS, E)
                units = []
                for c4 in range(KPJ):
                    for fs in range(E // FS):

                        def run(
                            ct=jj * KPJ + c4,
                            fsl=slice(fs * FS, (fs + 1) * FS),
                        ):
                            ps = psMM.tile([P, FS], F32, tag="mm")
                            for jt in range(JT):
                                nc.tensor.matmul(
                                    ps[:],
                                    hdt[:, jt, ct * P : (ct + 1) * P],
                                    wo[:, jt, fsl],
                                    start=(jt == 0),
                                    stop=(jt == JT - 1),
                                )
                            ysb = pT.tile([P, FS], BF, tag="ysb")
                            nc.vector.tensor_copy(ysb[:], ps[:])
                            nc.sync.dma_start(y_d[ct, :, fsl], ysb[:])

                        units.append(run)
                return units

            for u in proj_units(0):
                u()

            for j in range(NJ):
                jsl = slice(j * CS, (j + 1) * CS)
                nkt = (j + 1) * KPJ  # kk-tiles needed (causal)
                fillers = []
                if j + 1 < NJ:
                    load_x(j + 1)
                    fillers += proj_units(j + 1)
                if j >= 1:
                    fillers += outproj_units(j - 1)
                ngrps = JT * (j + 1)
                gdone = fi = 0
                norms = []

                def emit_norm(g, base, hds, jsl=jsl):
                    for half in range(2):

                        def run(
                            g=g,
                            row=base + 32 * half,
                            hd=hds[half],
                            half=half,
                            dt_g=den if g < 2 else den2,
                        ):
                            bct = psPV.tile(
                                [D + 1, CS], F32, tag="pv", name="bc"
                            )
                            bc = bct[0:64, :]
                            nc.tensor.matmul(
                                bc,
                                ones[row : row + 1, :],
                                dt_g[row : row + 1, :],
                                start=True,
                                stop=True,
                                tile_position=(row, 0),
                            )
                            if half == 0:
                                nc.vector.tensor_tensor(
                                    hdt[0:64, g, jsl], hd[:], bc, MUL
                                )
                            else:
                                tmp = pT.tile([64, CS], BF, tag="tmp")
                                nc.vector.tensor_tensor(
                                    tmp[:], hd[:], bc, MUL
                                )
                                nc.sync.dma_start(
                                    hdt[64:128, g, jsl], tmp[:]
                                )

                        norms.append(run)

                for g in range(JT):
                    pv_ps = [
                        psPV.tile([D + 1, CS], F32, tag="pv", name=f"pv{h}")
                        for h in range(2)
                    ]
                    # kk-tiles in groups of 4 (two 2-kt psum chunks) so the
                    # S^T matmuls and the PV accumulation run as longer
                    # back-to-back chains on the PE
                    for grp in range((nkt + 3) // 4):
                        group = []  # (kts, ws, s_ps, e_sb) per 2-kt chunk
                        for ck in (2 * grp, 2 * grp + 1):
                            kts = [k for k in (2 * ck, 2 * ck + 1) if k < nkt]
                            if not kts:
                                continue
                            ws = [max(0, k * P - j * CS) for k in kts]
                            s_ps = [
                                psS.tile([P, 2, CS], F32, tag="s", name=f"s{h}")
                                for h in range(2)
                            ]
                            e_sb = [
                                pE.tile([P, 2, CS], BF, tag="e", name=f"e{h}")
                                for h in range(2)
                            ]
                            group.append((kts, ws, s_ps, e_sb))
                            for i, kkt in enumerate(kts):
                                w = ws[i]
                                ksl = slice(kkt * P, (kkt + 1) * P)
                                qsl = slice(j * CS + w, (j + 1) * CS)
                                for half, base in ((0, 0), (1, 64)):
                                    nc.tensor.matmul(
                                        s_ps[half][:, i, w:CS],
                                        kt[base : base + 64, g, ksl],
                                        qt[base : base + 64, g, qsl],
                                        start=True,
                                        stop=True,
                                        tile_position=(base, 0),
                                    )
                        for kts, ws, s_ps, e_sb in group:
                            nck = len(kts)
                            wm = ws[0]
                            for half in range(2):
                                nc.scalar.activation(
                                    e_sb[half][:, 0:nck, wm:CS],
                                    s_ps[half][:, 0:nck, wm:CS],
                                    Exp,
                                    scale=scale,
                                )
                            for i, kkt in enumerate(kts):
                                w = ws[i]
                                if 0 <= kkt * P - j * CS < CS:
                                    for half in range(2):
                                        blk = e_sb[half][:, i, w : w + P]
                                        nc.gpsimd.affine_select(
                                            blk,
                                            blk,
                                            pattern=[[1, P]],
                                            compare_op=ISGE,
                                            fill=0.0,
                                            base=0,
                                            channel_multiplier=-1,
                                        )
                        for half in range(2):
                            h = 2 * g + half
                            for kts, ws, s_ps, e_sb in group:
                                for i, kkt in enumerate(kts):
                                    w = ws[i]
                                    nc.tensor.matmul(
                                        pv_ps[half][:, w:CS],
                                        v[:, kkt, h, :],
                                        e_sb[half][:, i, w:CS],
                                        start=(kkt == 0),
                                        stop=(kkt == nkt - 1),
                                    )
                        gdone += 1
                        # hold units back for the j-end normalize interleave
                        want = min(
                            len(fillers) * gdone // ngrps,
                            max(0, len(fillers) - 8),
                        )
                        while fi < want:
                            fillers[fi]()
                            fi += 1

                    # evict PV (bf16) + stage the two raw denominator rows to
                    # {base, base+32} of the shared den tile, reciprocal both
                    # in one DVE op, broadcast back via K=1 matmuls, multiply.
                    base = 64 * (g % 2)
                    dt_g = den if g < 2 else den2
                    for half in range(2):
                        row = base + 32 * half
                        if row == 64:
                            nc.vector.tensor_copy(
                                dt_g[64:65, :], pv_ps[half][D : D + 1, :]
                            )
                        else:
                            nc.vector.tensor_copy(
                                sg[64:65, half, :], pv_ps[half][D : D + 1, :]
                            )
                            nc.sync.dma_start(
                                dt_g[row : row + 1, :],
                                sg[64:65, half, :].bitcast(FR),
                            )
                    hds = []
                    for half in range(2):
                        hd = pH.tile([D, CS], BF, tag="hd", name=f"hd{half}")
                        nc.vector.tensor_copy(hd[:], pv_ps[half][0:D, :])
                        hds.append(hd)
                    with nc.allow_low_precision(
                        reason="fp32r reciprocal feeds fp32r matmul"
                    ):
                        nc.vector.reciprocal(
                            dt_g[base : base + 33, :],
                            dt_g[base : base + 33, :],
                        )
                    emit_norm(g, base, hds)

                # j-end: all reciprocals have finished (or are finishing)
                # behind the held-back fillers; interleave the broadcast +
                # normalize ops with them so the DVE multiply pacing never
                # idles the PE
                queue = []
                rest = fillers[fi:]
                fi = len(fillers)
                for a, b in zip(norms, rest):
                    queue += [a, b]
                longer = norms if len(norms) > len(rest) else rest
                queue += longer[min(len(norms), len(rest)) :]
                for u in queue:
                    u()

            for u in outproj_units(NJ - 1):
                u()
    return nc


def _split_waits_json(bir_json_bytes):
    """TRN2 TPB instructions have one sync-wait slot and this walrus build
    refuses to split multi-wait instructions, so hoist all but the last wait
    onto preceding wait-only EventSemaphore instructions (same engine,
    executed in order -> semantically identical)."""
    import json

    d = json.loads(bir_json_bytes)
    n = 0
    for fn in d["functions"]:
        for blk in fn["blocks"]:
            out = []
            for inst in blk["instructions"]:
                si = inst.get("sync_info")
                waits = (si or {}).get("on_wait") or []
                if len(waits) > 1:
                    for w in waits[:-1]:
                        n += 1
                        out.append(
                            {
                                "debug": inst.get("debug", 0),
                                "engine": inst["engine"],
                                "ins": [],
                                "name": f"wsplit-{n}",
                                "opcode": "EventSemaphore",
                                "outs": [],
                                "sync_info": {"on_update": [], "on_wait": [w]},
                            }
                        )
                    si["on_wait"] = [waits[-1]]
                out.append(inst)
            blk["instructions"] = out
    return json.dumps(d).encode()


def _striped(a, p=P):
    """[K, N] with K = kt*p + i  ->  contiguous [p, K//p, N]."""
    k, n = a.shape
    return np.ascontiguousarray(a.reshape(k // p, p, n).transpose(1, 0, 2))


def _bf16(a):
    import ml_dtypes

    return a.astype(ml_dtypes.bfloat16)


def prep_core_inputs(x_b, wq_s, wk_s, wv_s, wo_s):
    """Host-side layout prep for one core. x_b [C,E], w*_s column/row slices."""
    return {
        "xT": _bf16(_striped(np.ascontiguousarray(x_b.T))),
        "wq": _bf16(_striped(wq_s)),
        "wk": _bf16(_striped(wk_s)),
        "wv": _bf16(_striped(wv_s)),
        "wo": _bf16(_striped(wo_s)),
    }


_module_cache = {}


def kernel(x, W_q, W_k, W_v, W_o):
    from concourse.bass_utils import run_bass_kernel_spmd

    x = np.asarray(x, dtype=np.float32)
    W_q = np.asarray(W_q, dtype=np.float32)
    W_k = np.asarray(W_k, dtype=np.float32)
    W_v = np.asarray(W_v, dtype=np.float32)
    W_o = np.asarray(W_o, dtype=np.float32)

    HD2 = H * D // 2  # columns per head-group (512)
    in_maps = []
    for core in range(NCORES):
        b, hg = core // 2, core % 2
        cols = slice(hg * HD2, (hg + 1) * HD2)
        in_maps.append(
            prep_core_inputs(
                x[b], W_q[:, cols], W_k[:, cols], W_v[:, cols], W_o[cols, :]
            )
        )

    if "nc" not in _module_cache:
        nc = build_module()
        fixed = _split_waits_json(nc.to_json_bytes())
        nc.to_json_bytes = lambda: fixed
        _module_cache["nc"] = nc
    nc = _module_cache["nc"]

    res = run_bass_kernel_spmd(nc, in_maps, core_ids=list(range(NCORES)))
    _module_cache["last_res"] = res
    out = np.empty((B, C, E), dtype=np.float32)
    for b in range(B):
        ya = res.results[2 * b]["y"].astype(np.float32).reshape(C, E)
        yb = res.results[2 * b + 1]["y"].astype(np.float32).reshape(C, E)
        out[b] = ya + yb
    return out


if __name__ == "__main__":
    rng = np.random.default_rng(0)
    ins = {
        "x": rng.standard_normal((B, C, E), dtype=np.float32),
        "W_q": rng.standard_normal((E, H * D), dtype=np.float32) * 0.02,
        "W_k": rng.standard_normal((E, H * D), dtype=np.float32) * 0.02,
        "W_v": rng.standard_normal((E, H * D), dtype=np.float32) * 0.02,
        "W_o": rng.standard_normal((H * D, E), dtype=np.float32) * 0.02,
    }
    out = kernel(**ins)
    print("kernel ran, out shape", out.shape, "mean", out.mean())
